# revision 16
# baseline (speedup 1.0000x reference)
"""Censored-loss kernel for Trainium2, data-parallel over 8 NeuronCores.

Math (per reference):
    per_t = targets.sum(-1)                      # [B, T]
    mask  = prefix mask: mask[t] = 1 iff any per_t[t'] > 0 for t' >= t
    censor_p = 1 - outputs.sum(-1)
    loss  = sum(mask * (targets[:,:,0]*ln(censor_p+eps)
                        + sum_v targets[:,:,1+v]*ln(outputs[:,:,v]+eps)))
    count = sum(mask)
    result = -loss / max(count, 1)   (0 if count == 0)

Key structural ideas (targets >= 0 by construction; masked-out positions
have targets == 0 exactly, so they contribute nothing to loss or count):

  * Valid-length sorting + truncation: rows are sorted by valid-prefix
    length and packed into 128-row tiles truncated to the tile max length.
    Positions beyond a row's length have targets == 0 and drop out of both
    loss and count, so truncation is exact and halves work on every engine.
  * Planes layout, separate arenas: outputs planes [o0..o3] in one SBUF
    arena, target planes [t0..t4] in another; everything is a contiguous
    step-1 fp16 access -> DVE 2x packed mode, and Ln batches over several
    tiles in one ACT instruction (amortizes the ~224-cycle ACT overhead).
  * 4 large DMAs per arena (~2.5 MB each) instead of 16 small ones -> near
    peak HBM bandwidth, overlapping compute of earlier pieces.
  * censor sum via halves trick: s = (o0|o1) + (o2|o3), then fold.
  * prod_c = t0*lc and prod_v = t[1:]*lv as fp16 TTs (2x); PE ones-matmuls
    reduce chunks (<=512) into accumulating PSUM banks (pre-zeroed with a
    full-width start matmul so variable-width accumulation is safe).
    (The fused DVE TENSOR_TENSOR_REDUCE would avoid PE entirely but
    crashes the NRT runtime on this platform.)
  * count via tensor_scalar is_gt (4x mode) + PE matmul into a third bank.
  * Final reduction of the [1, T] PSUM partials happens on host in f64.
"""

import sys

if "/opt/trn_rl_repo" not in sys.path:
    sys.path.insert(0, "/opt/trn_rl_repo")

import numpy as np

import concourse.bacc as bacc
import concourse.mybir as mybir
import concourse.tile as tile
from concourse.bass_utils import run_bass_kernel_spmd

N_CORES = 8
B, T, V = 16384, 512, 5
P = 128                       # SBUF partitions
NTILES = (B // N_CORES) // P  # tiles (slots) per core
def piece_bounds(ntiles):
    """Piece boundaries over descending-width slots: big DMA pieces early
    (stream efficiency), small pieces at the end (short drain tail)."""
    bounds = []
    i = 0
    while i < ntiles:
        n = 4 if ntiles - i > 6 else 2
        n = min(n, ntiles - i)
        bounds.append((i, i + n))
        i += n
    return bounds
EPS = 1e-8
F32 = mybir.dt.float32
F16 = mybir.dt.float16
NPF16 = np.float16
ACT = mybir.ActivationFunctionType
ALU = mybir.AluOpType


def build_nc(widths):
    """widths: tuple of per-slot tile widths (multiples of 8, <= T)."""
    ntiles = len(widths)
    SW = sum(widths)
    oo = np.concatenate([[0], np.cumsum([4 * w for w in widths])])
    to = np.concatenate([[0], np.cumsum([5 * w for w in widths])])
    so = np.concatenate([[0], np.cumsum(widths)])

    nc = bacc.Bacc("TRN2", debug=False, num_devices=N_CORES)
    o_d = nc.dram_tensor("o_in", [P, 4 * SW], F16, kind="ExternalInput")
    t_d = nc.dram_tensor("t_in", [P, 5 * SW], F16, kind="ExternalInput")
    loss_d = nc.dram_tensor("loss_acc", [1, 2 * T], F32, kind="ExternalOutput")
    cnt_d = nc.dram_tensor("cnt_acc", [1, T], F32, kind="ExternalOutput")

    n_loss_mm = sum(-(-5 * w // T) for w in widths)

    with tile.TileContext(nc) as tc:
        with (
            tc.tile_pool(name="ar", bufs=1) as ar,
            tc.tile_pool(name="mid", bufs=5) as mid,
            tc.tile_pool(name="ps", bufs=1, space="PSUM") as psp,
        ):
            # persistent arenas
            O_a = ar.tile([P, 4 * SW], F16)
            T_a = ar.tile([P, 5 * SW], F16)
            LV_a = ar.tile([P, 4 * SW], F16)
            LC_a = ar.tile([P, SW], F16)
            S_a = ar.tile([P, SW], F16)
            SGN_a = ar.tile([P, SW], F16)
            eps_b = ar.tile([P, 1], F32)
            nc.vector.memset(eps_b[:], EPS)
            ones = ar.tile([P, 1], F16)
            nc.vector.memset(ones[:], 1.0)
            zt = ar.tile([P, T], F16)
            nc.vector.memset(zt[:], 0.0)

            loss_ps0 = psp.tile([1, T], F32, tag="lps0")
            loss_ps1 = psp.tile([1, T], F32, tag="lps1")
            loss_ps = [loss_ps0, loss_ps1]
            cnt_ps = psp.tile([1, T], F32, tag="cps")

            # Zero all PSUM banks full-width so later variable-width
            # accumulating matmuls never add onto stale PSUM contents.
            nc.tensor.matmul(loss_ps0[:], ones[:], zt[:], start=True, stop=False)
            nc.tensor.matmul(loss_ps1[:], ones[:], zt[:], start=True, stop=False)
            nc.tensor.matmul(cnt_ps[:], ones[:], zt[:], start=True, stop=False)

            nmm = 0
            cchunk = 0  # count-matmul progress in SGN arena (in cols)
            for lo, hi in piece_bounds(ntiles):
                # large DMA pieces into the arenas
                nc.sync.dma_start(
                    O_a[:][:, oo[lo] : oo[hi]], o_d.ap()[:, oo[lo] : oo[hi]]
                )
                nc.sync.dma_start(
                    T_a[:][:, to[lo] : to[hi]], t_d.ap()[:, to[lo] : to[hi]]
                )
                # censor sums for each tile in the piece
                for i in range(lo, hi):
                    w = widths[i]
                    ob = O_a[:][:, oo[i] : oo[i + 1]]
                    s2 = mid.tile([P, 2 * T], F16, tag="s2")
                    nc.vector.tensor_tensor(
                        s2[:][:, 0 : 2 * w], ob[:, 0 : 2 * w], ob[:, 2 * w : 4 * w],
                        op=ALU.add,
                    )
                    nc.vector.tensor_tensor(
                        S_a[:][:, so[i] : so[i + 1]],
                        s2[:][:, 0:w], s2[:][:, w : 2 * w], op=ALU.add,
                    )
                # batched Ln over the whole piece (one ACT instr each)
                nc.scalar.activation(
                    LV_a[:][:, oo[lo] : oo[hi]], O_a[:][:, oo[lo] : oo[hi]],
                    ACT.Ln, bias=eps_b[:],
                )
                nc.scalar.activation(
                    LC_a[:][:, so[lo] : so[hi]], S_a[:][:, so[lo] : so[hi]],
                    ACT.Ln, bias=1.0, scale=-1.0,
                )
                # products + PE reduction per tile
                for i in range(lo, hi):
                    w = widths[i]
                    t0 = T_a[:][:, to[i] : to[i] + w]
                    t4 = T_a[:][:, to[i] + w : to[i + 1]]

                    # single scr buffer [c | v] so loss matmuls chunk a
                    # contiguous [0:5w] region (fewest matmuls)
                    scr = mid.tile([P, 5 * T], F16, tag="scr")
                    nc.vector.tensor_tensor(
                        scr[:][:, 0:w], t0, LC_a[:][:, so[i] : so[i + 1]],
                        op=ALU.mult,
                    )
                    nc.vector.tensor_tensor(
                        scr[:][:, w : 5 * w], t4, LV_a[:][:, oo[i] : oo[i + 1]],
                        op=ALU.mult,
                    )
                    # count mask into persistent arena; reduced in batched
                    # 512-wide matmuls as chunks complete
                    nc.vector.tensor_scalar(
                        out=SGN_a[:][:, so[i] : so[i + 1]], in0=t0,
                        scalar1=0.0, scalar2=None, op0=ALU.is_gt,
                    )

                    c0 = 0
                    while c0 < 5 * w:
                        n = min(T, 5 * w - c0)
                        nc.tensor.matmul(
                            loss_ps[nmm % 2][:][:, 0:n],
                            ones[:],
                            scr[:][:, c0 : c0 + n],
                            start=False,
                            stop=(nmm >= n_loss_mm - 2),
                        )
                        nmm += 1
                        c0 += n
                    while so[i + 1] - cchunk >= T or (
                        i == ntiles - 1 and so[i + 1] > cchunk
                    ):
                        n = min(T, so[i + 1] - cchunk)
                        nc.tensor.matmul(
                            cnt_ps[:][:, 0:n],
                            ones[:],
                            SGN_a[:][:, cchunk : cchunk + n],
                            start=False,
                            stop=(cchunk + n == so[ntiles]),
                        )
                        cchunk += n

            loss_sb = ar.tile([1, 2 * T], F32)
            nc.scalar.copy(loss_sb[:, 0:T], loss_ps0[:])
            nc.scalar.copy(loss_sb[:, T : 2 * T], loss_ps1[:])
            cnt_sb = ar.tile([1, T], F32)
            nc.scalar.copy(cnt_sb[:], cnt_ps[:])
            nc.sync.dma_start(loss_d.ap(), loss_sb[:])
            nc.sync.dma_start(cnt_d.ap(), cnt_sb[:])
    nc.compile()
    return nc


_NC_CACHE = {}


def _get_nc(widths):
    if widths not in _NC_CACHE:
        _NC_CACHE[widths] = build_nc(widths)
    return _NC_CACHE[widths]


def pack_inputs(outputs, targets):
    """Sort rows by valid length, pack per-core planes layout, fp16."""
    outputs = np.asarray(outputs)
    targets = np.asarray(targets)
    nzmask = (targets != 0).any(axis=2)
    has = nzmask.any(axis=1)
    lengths = np.where(has, T - nzmask[:, ::-1].argmax(axis=1), 0)
    order = np.argsort(lengths, kind="stable")

    widths0 = []
    for i in range(NTILES):
        blk = order[P * N_CORES * i : P * N_CORES * (i + 1)]
        wi = int(lengths[blk].max()) if len(blk) else 8
        widths0.append(int(min(T, max(8, ((wi + 7) // 8) * 8))))

    # processing order: widest first.  The DMA stream paces the kernel, so
    # ending with the narrowest tiles minimizes the compute drain tail.
    perm = sorted(range(NTILES), key=lambda i: -widths0[i])
    widths = tuple(widths0[p] for p in perm)

    SW = sum(widths)
    O = np.zeros((N_CORES, P, 4 * SW), dtype=NPF16)
    TG = np.zeros((N_CORES, P, 5 * SW), dtype=NPF16)
    ooff = 0
    toff = 0
    for j, w in enumerate(widths):
        p = perm[j]
        for k in range(N_CORES):
            rows = order[P * (N_CORES * p + k) : P * (N_CORES * p + k) + P]
            o_blk = outputs[rows, :w, :].transpose(0, 2, 1).reshape(P, 4 * w)
            t_blk = targets[rows, :w, :].transpose(0, 2, 1).reshape(P, 5 * w)
            O[k, :, ooff : ooff + 4 * w] = o_blk
            TG[k, :, toff : toff + 5 * w] = t_blk
        ooff += 4 * w
        toff += 5 * w
    return O, TG, widths


def run_spmd(outputs, targets, trace=False, **kwargs):
    O, TG, widths = pack_inputs(outputs, targets)
    in_maps = [{"o_in": O[k], "t_in": TG[k]} for k in range(N_CORES)]
    nc = _get_nc(widths)
    res = run_bass_kernel_spmd(
        nc, in_maps, core_ids=list(range(N_CORES)), trace=trace, **kwargs
    )
    loss = sum(r["loss_acc"].astype(np.float64).sum() for r in res.results)
    cnt = sum(r["cnt_acc"].astype(np.float64).sum() for r in res.results)
    return loss, cnt, res


def kernel(outputs, targets):
    loss, cnt, _ = run_spmd(outputs, targets)
    if cnt > 0:
        return np.float32(-loss / max(cnt, 1.0))
    return np.float32(0.0)


# revision 28
# speedup vs baseline: 1.0710x; 1.0710x over previous
"""Censored-loss kernel for Trainium2, data-parallel over 8 NeuronCores.

Math (per reference):
    per_t = targets.sum(-1)                      # [B, T]
    mask  = prefix mask: mask[t] = 1 iff any per_t[t'] > 0 for t' >= t
    censor_p = 1 - outputs.sum(-1)
    loss  = sum(mask * (targets[:,:,0]*ln(censor_p+eps)
                        + sum_v targets[:,:,1+v]*ln(outputs[:,:,v]+eps)))
    count = sum(mask)
    result = -loss / max(count, 1)   (0 if count == 0)

Key structural ideas (targets >= 0 by construction; masked-out positions
have targets == 0 exactly, so they contribute nothing to loss or count):

  * Valid-length sorting + truncation: rows are sorted by valid-prefix
    length and packed into 128-row tiles truncated to the tile max length.
    Positions beyond a row's length have targets == 0 and drop out of both
    loss and count, so truncation is exact and halves work on every engine.
  * Planes layout, separate arenas: outputs planes [o0..o3] in one SBUF
    arena, target planes [t0..t4] in another; everything is a contiguous
    step-1 fp16 access -> DVE 2x packed mode, and Ln batches over several
    tiles in one ACT instruction (amortizes the ~224-cycle ACT overhead).
  * 4 large DMAs per arena (~2.5 MB each) instead of 16 small ones -> near
    peak HBM bandwidth, overlapping compute of earlier pieces.
  * censor sum via halves trick: s = (o0|o1) + (o2|o3), then fold.
  * prod_c = t0*lc and prod_v = t[1:]*lv as fp16 TTs (2x); PE ones-matmuls
    reduce chunks (<=512) into accumulating PSUM banks (pre-zeroed with a
    full-width start matmul so variable-width accumulation is safe).
    (The fused DVE TENSOR_TENSOR_REDUCE would avoid PE entirely but
    crashes the NRT runtime on this platform.)
  * count via tensor_scalar is_gt (4x mode) + PE matmul into a third bank.
  * Final reduction of the [1, T] PSUM partials happens on host in f64.
"""

import sys

if "/opt/trn_rl_repo" not in sys.path:
    sys.path.insert(0, "/opt/trn_rl_repo")

import numpy as np

import concourse.bacc as bacc
import concourse.mybir as mybir
import concourse.tile as tile
from concourse.bass_utils import run_bass_kernel_spmd

N_CORES = 8
B, T, V = 16384, 512, 5
P = 128                       # SBUF partitions
NTILES = (B // N_CORES) // P  # tiles (slots) per core
def piece_bounds(ntiles):
    """Pieces of 2 slots each."""
    return [(i, min(i + 2, ntiles)) for i in range(0, ntiles, 2)]
EPS = 1e-8
F32 = mybir.dt.float32
F16 = mybir.dt.float16
NPF16 = np.float16
ACT = mybir.ActivationFunctionType
ALU = mybir.AluOpType


def build_nc(widths):
    """widths: tuple of per-slot tile widths (multiples of 8, <= T)."""
    ntiles = len(widths)
    SW = sum(widths)
    oo = np.concatenate([[0], np.cumsum([4 * w for w in widths])])
    to = np.concatenate([[0], np.cumsum([5 * w for w in widths])])
    so = np.concatenate([[0], np.cumsum(widths)])

    nc = bacc.Bacc("TRN2", debug=False, num_devices=N_CORES)
    o_d = nc.dram_tensor("o_in", [P, 4 * SW], F16, kind="ExternalInput")
    t_d = nc.dram_tensor("t_in", [P, 5 * SW], F16, kind="ExternalInput")
    loss_d = nc.dram_tensor("loss_acc", [1, 4 * T], F32, kind="ExternalOutput")
    cnt_d = nc.dram_tensor("cnt_acc", [1, T], F32, kind="ExternalOutput")

    n_loss_mm = sum(-(-5 * w // T) for w in widths)
    NBANK = 4

    with tile.TileContext(nc) as tc:
        with (
            tc.tile_pool(name="ar", bufs=1) as ar,
            tc.tile_pool(name="mid", bufs=5) as mid,
            tc.tile_pool(name="ps", bufs=1, space="PSUM") as psp,
        ):
            # persistent arenas
            O_a = ar.tile([P, 4 * SW], F16)
            T_a = ar.tile([P, 5 * SW], F16)
            LV_a = ar.tile([P, 4 * SW], F16)
            LC_a = ar.tile([P, SW], F16)
            S_a = ar.tile([P, SW], F16)
            SGN_a = ar.tile([P, SW], F16)
            eps_b = ar.tile([P, 1], F32)
            nc.vector.memset(eps_b[:], EPS)
            ones = ar.tile([P, 1], F16)
            nc.vector.memset(ones[:], 1.0)
            zt = ar.tile([P, T], F16)
            nc.vector.memset(zt[:], 0.0)

            loss_ps = [psp.tile([1, T], F32, tag=f"lps{j}", name=f"lps{j}")
                       for j in range(NBANK)]
            cnt_ps = psp.tile([1, T], F32, tag="cps")

            # Zero all PSUM banks full-width so later variable-width
            # accumulating matmuls never add onto stale PSUM contents.
            for j in range(NBANK):
                nc.tensor.matmul(
                    loss_ps[j][:], ones[:], zt[:], start=True, stop=False
                )
            nc.tensor.matmul(cnt_ps[:], ones[:], zt[:], start=True, stop=False)

            nmm = 0
            cchunk = 0  # count-matmul progress in SGN arena (in cols)
            for lo, hi in piece_bounds(ntiles):
                # large DMA pieces into the arenas
                nc.sync.dma_start(
                    O_a[:][:, oo[lo] : oo[hi]], o_d.ap()[:, oo[lo] : oo[hi]]
                )
                nc.sync.dma_start(
                    T_a[:][:, to[lo] : to[hi]], t_d.ap()[:, to[lo] : to[hi]]
                )
                # censor sums for each tile in the piece
                for i in range(lo, hi):
                    w = widths[i]
                    ob = O_a[:][:, oo[i] : oo[i + 1]]
                    s2 = mid.tile([P, 2 * T], F16, tag="s2")
                    nc.vector.tensor_tensor(
                        s2[:][:, 0 : 2 * w], ob[:, 0 : 2 * w], ob[:, 2 * w : 4 * w],
                        op=ALU.add,
                    )
                    nc.vector.tensor_tensor(
                        S_a[:][:, so[i] : so[i + 1]],
                        s2[:][:, 0:w], s2[:][:, w : 2 * w], op=ALU.add,
                    )
                # batched Ln over the whole piece (one ACT instr each)
                nc.scalar.activation(
                    LV_a[:][:, oo[lo] : oo[hi]], O_a[:][:, oo[lo] : oo[hi]],
                    ACT.Ln, bias=eps_b[:],
                )
                nc.scalar.activation(
                    LC_a[:][:, so[lo] : so[hi]], S_a[:][:, so[lo] : so[hi]],
                    ACT.Ln, bias=1.0, scale=-1.0,
                )
                # products + PE reduction per tile
                for i in range(lo, hi):
                    w = widths[i]
                    t0 = T_a[:][:, to[i] : to[i] + w]
                    t4 = T_a[:][:, to[i] + w : to[i + 1]]

                    # single scr buffer [c | v] so loss matmuls chunk a
                    # contiguous [0:5w] region (fewest matmuls)
                    scr = mid.tile([P, 5 * T], F16, tag="scr")
                    nc.vector.tensor_tensor(
                        scr[:][:, 0:w], t0, LC_a[:][:, so[i] : so[i + 1]],
                        op=ALU.mult,
                    )
                    nc.vector.tensor_tensor(
                        scr[:][:, w : 5 * w], t4, LV_a[:][:, oo[i] : oo[i + 1]],
                        op=ALU.mult,
                    )
                    # count mask into persistent arena; reduced in batched
                    # 512-wide matmuls as chunks complete
                    nc.vector.tensor_scalar(
                        out=SGN_a[:][:, so[i] : so[i + 1]], in0=t0,
                        scalar1=0.0, scalar2=None, op0=ALU.is_gt,
                    )

                    c0 = 0
                    while c0 < 5 * w:
                        n = min(T, 5 * w - c0)
                        nc.tensor.matmul(
                            loss_ps[nmm % NBANK][:][:, 0:n],
                            ones[:],
                            scr[:][:, c0 : c0 + n],
                            start=False,
                            stop=(nmm >= n_loss_mm - NBANK),
                        )
                        nmm += 1
                        c0 += n
                    while so[i + 1] - cchunk >= T or (
                        i == ntiles - 1 and so[i + 1] > cchunk
                    ):
                        n = min(T, so[i + 1] - cchunk)
                        nc.tensor.matmul(
                            cnt_ps[:][:, 0:n],
                            ones[:],
                            SGN_a[:][:, cchunk : cchunk + n],
                            start=False,
                            stop=(cchunk + n == so[ntiles]),
                        )
                        cchunk += n

            loss_sb = ar.tile([1, 4 * T], F32)
            for j in range(NBANK):
                nc.scalar.copy(loss_sb[:, j * T : (j + 1) * T], loss_ps[j][:])
            cnt_sb = ar.tile([1, T], F32)
            nc.scalar.copy(cnt_sb[:], cnt_ps[:])
            nc.sync.dma_start(loss_d.ap(), loss_sb[:])
            nc.sync.dma_start(cnt_d.ap(), cnt_sb[:])
    nc.compile()
    return nc


_NC_CACHE = {}


def _get_nc(widths):
    if widths not in _NC_CACHE:
        _NC_CACHE[widths] = build_nc(widths)
    return _NC_CACHE[widths]


def pack_inputs(outputs, targets):
    """Sort rows by valid length, pack per-core planes layout, fp16."""
    outputs = np.asarray(outputs)
    targets = np.asarray(targets)
    nzmask = (targets != 0).any(axis=2)
    has = nzmask.any(axis=1)
    lengths = np.where(has, T - nzmask[:, ::-1].argmax(axis=1), 0)
    order = np.argsort(lengths, kind="stable")

    widths0 = []
    for i in range(NTILES):
        blk = order[P * N_CORES * i : P * N_CORES * (i + 1)]
        wi = int(lengths[blk].max()) if len(blk) else 8
        widths0.append(int(min(T, max(8, ((wi + 7) // 8) * 8))))

    # processing order interleaves narrow and wide tiles so every
    # piece-sized pipeline stage carries roughly equal work
    perm = []
    lo_i, hi_i = 0, NTILES - 1
    while lo_i <= hi_i:
        perm.append(lo_i)
        if hi_i != lo_i:
            perm.append(hi_i)
        lo_i += 1
        hi_i -= 1
    widths = tuple(widths0[p] for p in perm)

    SW = sum(widths)
    O = np.zeros((N_CORES, P, 4 * SW), dtype=NPF16)
    TG = np.zeros((N_CORES, P, 5 * SW), dtype=NPF16)
    ooff = 0
    toff = 0
    for j, w in enumerate(widths):
        p = perm[j]
        for k in range(N_CORES):
            rows = order[P * (N_CORES * p + k) : P * (N_CORES * p + k) + P]
            o_blk = outputs[rows, :w, :].transpose(0, 2, 1).reshape(P, 4 * w)
            t_blk = targets[rows, :w, :].transpose(0, 2, 1).reshape(P, 5 * w)
            O[k, :, ooff : ooff + 4 * w] = o_blk
            TG[k, :, toff : toff + 5 * w] = t_blk
        ooff += 4 * w
        toff += 5 * w
    return O, TG, widths


def run_spmd(outputs, targets, trace=False, **kwargs):
    O, TG, widths = pack_inputs(outputs, targets)
    in_maps = [{"o_in": O[k], "t_in": TG[k]} for k in range(N_CORES)]
    nc = _get_nc(widths)
    res = run_bass_kernel_spmd(
        nc, in_maps, core_ids=list(range(N_CORES)), trace=trace, **kwargs
    )
    loss = sum(r["loss_acc"].astype(np.float64).sum() for r in res.results)
    cnt = sum(r["cnt_acc"].astype(np.float64).sum() for r in res.results)
    return loss, cnt, res


def kernel(outputs, targets):
    loss, cnt, _ = run_spmd(outputs, targets)
    if cnt > 0:
        return np.float32(-loss / max(cnt, 1.0))
    return np.float32(0.0)
